# revision 1
# baseline (speedup 1.0000x reference)
"""Cosine attention (B=1, H=16, S=4096, D=64) on 8 trn2 NeuronCores.

Sharding: batch*heads split across cores -> 2 heads per core, full seq per
head (softmax is per-row, no cross-core communication).

Per-head pipeline on a core:
  1. load Q,K,V [4096,64] in "(p t) d" layout -> [128, 32, 64]
  2. row sumsq (ACT Square + DVE reduce), rsqrt via Newton iterations (DVE
     bit-trick, no ACT table switches), fold exp(min(logit_scale, ln 8))
     into the Q-side scale
  3. normalize (broadcast tensor_tensor, fp16) + DMA-xbar transpose to
     d-major fp16: QT2 [128, 32, 128] (partition halves both hold Qt),
     KT2 [128, 16, 128] (even chunks in partitions 0-63, odd in 64-127)
  4. stage 1: row-packed fp16 matmul pairs S^T[j-chunk, i-block] -> PSUM
     slabs [128, 1536] (ping-pong)
  5. exp on ScalarE straight from PSUM -> E^T tiles (fp16)
  6. stage 2 (fp16): o_ps[65, 512] += Vx[chunk].T @ E^T (Vx has a ones
     column so row 64 accumulates the softmax denominator Z)
  7. drain: copy to SBUF, PE-transpose back to row-major, multiply by
     1/Z (fast reciprocal), one 1 MB DMA out per head.
"""

import math
from contextlib import ExitStack

import numpy as np

import concourse.bass as bass
import concourse.tile as tile
from concourse import bacc, mybir
import concourse.bass_utils as bass_utils
from concourse.masks import make_identity

F32 = mybir.dt.float32
F32R = mybir.dt.float32r
BF16 = mybir.dt.bfloat16
FP16 = mybir.dt.float16
I32 = mybir.dt.int32

N_CORES = 8
H_TOTAL = 16
H_PER_CORE = H_TOTAL // N_CORES
D = 64
IBW = 512          # i-block width (PSUM bank / fp32 moving-operand limit)
SLABW = 1536       # exp slab width (3 PSUM banks)


def _newton_rsqrt(nc, pool, ss, n):
    """rsqrt of ss [128, n] (fp32, positive) via bit-trick seed + 3 Newton
    iterations, all on VectorE. Returns a [128, n] fp32 AP."""
    seed_i = pool.tile([128, n], I32, tag="nw_i")
    # ~(i >> 1)
    nc.vector.tensor_scalar(
        out=seed_i[:], in0=ss.bitcast(I32), scalar1=1, scalar2=-1,
        op0=mybir.AluOpType.logical_shift_right, op1=mybir.AluOpType.bitwise_xor)
    # + (0x5f3759df + 1)  == 0x5f3759df - (i >> 1)
    nc.vector.tensor_scalar(
        out=seed_i[:], in0=seed_i[:], scalar1=0x5F3759E0, scalar2=None,
        op0=mybir.AluOpType.add)
    y = seed_i.bitcast(F32)
    t = pool.tile([128, n], F32, tag="nw_t")
    for _ in range(3):
        nc.vector.tensor_mul(t[:], y, y)            # y*y
        nc.vector.tensor_mul(t[:], t[:], ss)        # x*y*y
        nc.vector.tensor_scalar(                    # 1.5 - 0.5*x*y*y
            out=t[:], in0=t[:], scalar1=-0.5, scalar2=1.5,
            op0=mybir.AluOpType.mult, op1=mybir.AluOpType.add)
        nc.vector.tensor_mul(y, y, t[:])            # y *= t
    return y


def build_kernel(S):
    """Build the 2-head-per-core cosine attention program for seq len S."""
    NT = S // 128            # 128-row tiles per head
    NPAIR = NT // 2
    NIB = S // IBW           # i-blocks
    CPI = IBW // 128         # output chunks per i-block

    nc = bacc.Bacc("TRN2", target_bir_lowering=False, debug=False,
                   enable_asserts=False, num_devices=N_CORES)

    q_d = nc.dram_tensor("q", [H_PER_CORE, S, D], F32, kind="ExternalInput").ap()
    k_d = nc.dram_tensor("k", [H_PER_CORE, S, D], F32, kind="ExternalInput").ap()
    v_d = nc.dram_tensor("v", [H_PER_CORE, S, D], F32, kind="ExternalInput").ap()
    qs_d = nc.dram_tensor("qs", [128, H_PER_CORE], F32, kind="ExternalInput").ap()
    o_d = nc.dram_tensor("o", [H_PER_CORE, S, D], F32, kind="ExternalOutput").ap()

    # Each stage-1 matmul output S^T[j-chunk, i-block] is [128, IBW] = one
    # PSUM bank; a slab holds SLABW/IBW of them, exp'd by one ACT op.
    cpg = SLABW // IBW
    groups = []
    c = 0
    while c < NT:
        n = min(cpg, NT - c)
        groups.append((c, n))
        c += n

    with tile.TileContext(nc) as tc, ExitStack() as ctx:
        singles = ctx.enter_context(tc.tile_pool(name="singles", bufs=1))
        nat = ctx.enter_context(tc.tile_pool(name="nat", bufs=2))
        stats = ctx.enter_context(tc.tile_pool(name="stats", bufs=2))
        bigT = ctx.enter_context(tc.tile_pool(name="bigT", bufs=2))
        et_pool = ctx.enter_context(tc.tile_pool(name="et", bufs=6))
        osb_pool = ctx.enter_context(tc.tile_pool(name="osb", bufs=2))
        zr_pool = ctx.enter_context(tc.tile_pool(name="zr", bufs=4))
        out_pool = ctx.enter_context(tc.tile_pool(name="outp", bufs=2))

        ident = singles.tile([128, 128], F32)
        make_identity(nc, ident)
        qs_sb = singles.tile([128, H_PER_CORE], F32)
        nc.sync.dma_start(qs_sb[:], qs_d[:, :])

        QT2, KT2, VX = [], [], []

        # ---------------- PREP: both heads (no PE/PSUM use at all --
        # transposes go through the DMA xbar, so prep overlaps main freely).
        # K path is emitted first: every stage-1 matmul needs all of KT2,
        # while QT2 is consumed i-block by i-block.
        from concourse.tile_rust import add_dep_helper
        gate = None   # last critical DVE inst of head 0, ordering hint
        for h in range(H_PER_CORE):
            qh = q_d[h].rearrange("(p t) d -> p t d", t=NT)
            kh = k_d[h].rearrange("(p t) d -> p t d", t=NT)
            vh = v_d[h].rearrange("(p t) d -> p t d", t=NT)

            q_nat = nat.tile([128, NT, D], F32, tag="qnat")
            k_nat = nat.tile([128, NT, D], F32, tag="knat")
            nc.sync.dma_start(k_nat[:], kh)
            nc.sync.dma_start(q_nat[:], qh)

            qt2 = bigT.tile([128, NT, 128], FP16, tag="qt2")
            kt2 = bigT.tile([128, NPAIR, 128], FP16, tag="kt2")
            vx = bigT.tile([128, NT, D + 1], FP16, tag="vx")
            QT2.append(qt2)
            KT2.append(kt2)
            VX.append(vx)

            # V: fp16 via cast-DMA (SWDGE) + ones column
            nc.gpsimd.dma_start(vx[:, :, 0:D], vh)
            ones = stats.tile([128, NT], F32, tag="ones")
            nc.vector.memset(ones[:], 1.0)
            nc.vector.tensor_copy(
                vx[:, :, D:D + 1].rearrange("p t one -> p (t one)"), ones[:])

            # ---- K path ----
            sk = stats.tile([128, NT, D], F32, tag="sk")
            ssk = stats.tile([128, NT], F32, tag="ssk")
            if h == 0:
                nc.scalar.activation(sk[:], k_nat[:],
                                     mybir.ActivationFunctionType.Square)
            else:
                i0 = nc.vector.tensor_mul(sk[:], k_nat[:], k_nat[:])
                if gate is not None:
                    add_dep_helper(i0.ins, gate.ins, sync=False,
                                   reason="head1 prep after head0 critical path")
            nc.vector.tensor_reduce(
                ssk[:].rearrange("p (t one) -> p t one", one=1), sk[:],
                axis=mybir.AxisListType.X, op=mybir.AluOpType.add)
            rk = _newton_rsqrt(nc, stats, ssk[:], NT)
            kn_all = nat.tile([128, NT, D], FP16, tag="knall")
            nc.vector.tensor_mul(
                kn_all[:], k_nat[:],
                rk.rearrange("p (t one) -> p t one", one=1)
                .to_broadcast([128, NT, D]))
            for a4 in range(0, NPAIR, 8):
                na4 = min(8, NPAIR - a4)
                nc.sync.dma_start_transpose(
                    kt2[:, a4:a4 + na4, :],
                    kn_all[:, 2 * a4:2 * (a4 + na4), :].rearrange("p t d -> p (t d)"))

            # ---- Q path ----
            sq = stats.tile([128, NT, D], F32, tag="sq")
            ssq = stats.tile([128, NT], F32, tag="ssq")
            if h == 0:
                nc.scalar.activation(sq[:], q_nat[:],
                                     mybir.ActivationFunctionType.Square)
            else:
                nc.vector.tensor_mul(sq[:], q_nat[:], q_nat[:])
            nc.vector.tensor_reduce(
                ssq[:].rearrange("p (t one) -> p t one", one=1), sq[:],
                axis=mybir.AxisListType.X, op=mybir.AluOpType.add)
            rq = _newton_rsqrt(nc, stats, ssq[:], NT)
            # fold per-head logit scale into the q side
            nc.vector.tensor_scalar_mul(rq, rq, qs_sb[:, h:h + 1])
            qn2_all = nat.tile([128, NT, 2, D], FP16, tag="qnall")
            rq_b = (rq.rearrange("p (t one) -> p t one", one=1)
                    .to_broadcast([128, NT, D]))
            nc.vector.tensor_mul(qn2_all[:, :, 0, :], q_nat[:], rq_b)
            g = nc.vector.tensor_mul(qn2_all[:, :, 1, :], q_nat[:], rq_b)
            if h == 0:
                gate = g
            for t4 in range(0, NT, 8):
                nt4 = min(8, NT - t4)
                nc.sync.dma_start_transpose(
                    qt2[:, t4:t4 + nt4, :],
                    qn2_all[:, t4:t4 + nt4, :, :].rearrange("p t a d -> p (t a d)"))

        # ---------------- MAIN: per head ----------------
        ps_slab = ctx.enter_context(tc.tile_pool(name="ps_slab", bufs=2, space="PSUM"))
        ps_o = ctx.enter_context(tc.tile_pool(name="ps_o", bufs=1, space="PSUM"))
        ps_ot = ctx.enter_context(tc.tile_pool(name="ps_ot", bufs=1, space="PSUM"))
        def emit_drain(pend):
            """Drain a finished o_ps accumulator: copy to SBUF, PE-transpose
            each 128-column chunk back to row-major, multiply by 1/Z."""
            o_ps, out_sb, ib = pend
            o_sb = osb_pool.tile([65, IBW], F32, tag="osb")
            nc.vector.tensor_copy(o_sb[:], o_ps[:])
            for tchunk in range(CPI):
                otp = ps_ot.tile([128, D + 1], F32, tag="otp")
                nc.tensor.transpose(
                    otp[:], o_sb[:, tchunk * 128:(tchunk + 1) * 128],
                    ident[0:65, 0:65])
                zr = zr_pool.tile([128, 1], F32, tag="zrt")
                nc.vector.reciprocal_approx_fast(zr[:], otp[:, D:D + 1])
                nc.vector.tensor_scalar_mul(
                    out_sb[:, CPI * ib + tchunk, :], otp[:, 0:D], zr[:])

        pending = None
        OUT_SB = []
        for h in range(H_PER_CORE):
            qt2, kt2, vx = QT2[h], KT2[h], VX[h]
            out_sb = out_pool.tile([128, NT, D], F32, tag="outsb")
            OUT_SB.append(out_sb)
            for ib in range(NIB):
                rhsA = qt2[0:64, CPI * ib:CPI * (ib + 1), :].rearrange("p a b -> p (a b)")
                rhsB = qt2[64:128, CPI * ib:CPI * (ib + 1), :].rearrange("p a b -> p (a b)")
                o_ps = ps_o.tile([65, IBW], F32, tag="ops")
                for gi, (c0, ng) in enumerate(groups):
                    slab = ps_slab.tile([128, SLABW], F32, tag="slab")
                    for cc in range(ng):
                        c = c0 + cc
                        if c % 2 == 0:
                            nc.tensor.matmul(
                                slab[:, cc * IBW:(cc + 1) * IBW],
                                kt2[0:64, c // 2, :], rhsA,
                                start=True, stop=True, tile_position=(0, 0))
                        else:
                            nc.tensor.matmul(
                                slab[:, cc * IBW:(cc + 1) * IBW],
                                kt2[64:128, c // 2, :], rhsB,
                                start=True, stop=True, tile_position=(64, 0))
                    if gi == 0 and pending is not None:
                        # drain the previous i-block only after this one's
                        # first slab fill is queued, so PE never stalls on it
                        emit_drain(pending)
                        pending = None
                    et = et_pool.tile([128, SLABW], FP16, tag="et")
                    nc.scalar.activation(et[:, 0:ng * IBW], slab[:, 0:ng * IBW],
                                         mybir.ActivationFunctionType.Exp)
                    for cc in range(ng):
                        c = c0 + cc
                        nc.tensor.matmul(
                            o_ps[:], vx[:, c, :], et[:, cc * IBW:(cc + 1) * IBW],
                            start=(c == 0), stop=(c == NT - 1),
                            skip_group_check=True)
                pending = (o_ps, out_sb, ib)
            # out DMA emitted after the head's last drain (see below)
        emit_drain(pending)
        pending = None
        for h in range(H_PER_CORE):
            nc.sync.dma_start(
                o_d[h].rearrange("(p t) d -> p t d", t=NT), OUT_SB[h][:])

    nc.compile()
    return nc


_NC_CACHE = {}
TRACE = False        # set by test harness for profiling runs
LAST_RESULT = None   # BassKernelResults of the most recent kernel() call


def _get_nc(S):
    if S not in _NC_CACHE:
        _NC_CACHE[S] = build_kernel(S)
    return _NC_CACHE[S]


def kernel(queries, keys, values, logit_scale):
    B, H, S, D_ = queries.shape
    assert B == 1 and D_ == D and H == H_TOTAL
    nc = _get_nc(S)

    # host-side: per-head scale = exp(min(logit_scale, ln sqrt(D)))
    scale = np.exp(np.minimum(np.asarray(logit_scale, np.float32).reshape(H),
                              math.log(math.sqrt(D)))).astype(np.float32)

    in_maps = []
    for c in range(N_CORES):
        h0 = c * H_PER_CORE
        sl = slice(h0, h0 + H_PER_CORE)
        in_maps.append({
            "q": np.ascontiguousarray(queries[0, sl]),
            "k": np.ascontiguousarray(keys[0, sl]),
            "v": np.ascontiguousarray(values[0, sl]),
            "qs": np.ascontiguousarray(
                np.broadcast_to(scale[sl][None, :], (128, H_PER_CORE))),
        })

    res = bass_utils.run_bass_kernel_spmd(
        nc, in_maps, core_ids=list(range(N_CORES)), trace=TRACE)
    global LAST_RESULT
    LAST_RESULT = res

    out = np.empty((B, H, S, D), np.float32)
    for c in range(N_CORES):
        out[0, c * H_PER_CORE:(c + 1) * H_PER_CORE] = res.results[c]["o"]
    return out



# revision 4
# speedup vs baseline: 1.1017x; 1.1017x over previous
"""Cosine attention (B=1, H=16, S=4096, D=64) on 8 trn2 NeuronCores.

Sharding: batch*heads split across cores -> 2 heads per core, full seq per
head (softmax is per-row, no cross-core communication).

Per-head pipeline on a core:
  1. load Q,K,V [4096,64] in "(p t) d" layout -> [128, 32, 64]
  2. row sumsq (ACT Square + DVE reduce), rsqrt via Newton iterations (DVE
     bit-trick, no ACT table switches), fold exp(min(logit_scale, ln 8))
     into the Q-side scale
  3. normalize (broadcast tensor_tensor, fp16) + DMA-xbar transpose to
     d-major fp16: QT2 [128, 32, 128] (partition halves both hold Qt),
     KT2 [128, 16, 128] (even chunks in partitions 0-63, odd in 64-127)
  4. stage 1: row-packed fp16 matmul pairs S^T[j-chunk, i-block] -> PSUM
     slabs [128, 1536] (ping-pong)
  5. exp on ScalarE straight from PSUM -> E^T tiles (fp16)
  6. stage 2 (fp16): o_ps[65, 512] += Vx[chunk].T @ E^T (Vx has a ones
     column so row 64 accumulates the softmax denominator Z)
  7. drain: copy to SBUF, PE-transpose back to row-major, multiply by
     1/Z (fast reciprocal), one 1 MB DMA out per head.
"""

import math
from contextlib import ExitStack

import numpy as np

import concourse.bass as bass
import concourse.tile as tile
from concourse import bacc, mybir
import concourse.bass_utils as bass_utils
from concourse.masks import make_identity

F32 = mybir.dt.float32
F32R = mybir.dt.float32r
BF16 = mybir.dt.bfloat16
FP16 = mybir.dt.float16
I32 = mybir.dt.int32
I16 = mybir.dt.int16

N_CORES = 8
H_TOTAL = 16
H_PER_CORE = H_TOTAL // N_CORES
D = 64
IBW = 512          # i-block width (PSUM bank / fp32 moving-operand limit)
SLABW = 1536       # exp slab width (3 PSUM banks)

# Schraudolph fast-exp on DVE: i16 = round(x*A + B), bitcast as fp16 gives
# exp(x)*(1+eps), |eps|<~3.5%. Softmax cancels the common-mode part; the
# residual output error for a ~1/3 share of entries is ~8e-3 (simulated).
A_SCH = 1024.0 * 1.4426950408889634
B_SCH = 15.0 * 1024.0 - 45.0
DVE_GROUPS = (2, 5, 8)   # slab groups exp'd on DVE; group 10 is split


def _newton_rsqrt(nc, pool, ss, n):
    """rsqrt of ss [128, n] (fp32, positive) via bit-trick seed + 3 Newton
    iterations, all on VectorE. Returns a [128, n] fp32 AP."""
    seed_i = pool.tile([128, n], I32, tag="nw_i")
    # ~(i >> 1)
    nc.vector.tensor_scalar(
        out=seed_i[:], in0=ss.bitcast(I32), scalar1=1, scalar2=-1,
        op0=mybir.AluOpType.logical_shift_right, op1=mybir.AluOpType.bitwise_xor)
    # + (0x5f3759df + 1)  == 0x5f3759df - (i >> 1)
    nc.vector.tensor_scalar(
        out=seed_i[:], in0=seed_i[:], scalar1=0x5F3759E0, scalar2=None,
        op0=mybir.AluOpType.add)
    y = seed_i.bitcast(F32)
    t = pool.tile([128, n], F32, tag="nw_t")
    for _ in range(3):
        nc.vector.tensor_mul(t[:], y, y)            # y*y
        nc.vector.tensor_mul(t[:], t[:], ss)        # x*y*y
        nc.vector.tensor_scalar(                    # 1.5 - 0.5*x*y*y
            out=t[:], in0=t[:], scalar1=-0.5, scalar2=1.5,
            op0=mybir.AluOpType.mult, op1=mybir.AluOpType.add)
        nc.vector.tensor_mul(y, y, t[:])            # y *= t
    return y


def build_kernel(S):
    """Build the 2-head-per-core cosine attention program for seq len S."""
    NT = S // 128            # 128-row tiles per head
    NPAIR = NT // 2
    NIB = S // IBW           # i-blocks
    CPI = IBW // 128         # output chunks per i-block

    nc = bacc.Bacc("TRN2", target_bir_lowering=False, debug=False,
                   enable_asserts=False, num_devices=N_CORES)

    q_d = nc.dram_tensor("q", [H_PER_CORE, S, D], F32, kind="ExternalInput").ap()
    k_d = nc.dram_tensor("k", [H_PER_CORE, S, D], F32, kind="ExternalInput").ap()
    v_d = nc.dram_tensor("v", [H_PER_CORE, S, D], F32, kind="ExternalInput").ap()
    qs_d = nc.dram_tensor("qs", [128, H_PER_CORE], F32, kind="ExternalInput").ap()
    o_d = nc.dram_tensor("o", [H_PER_CORE, S, D], F32, kind="ExternalOutput").ap()

    # Each stage-1 matmul output S^T[j-chunk, i-block] is [128, IBW] = one
    # PSUM bank; a slab holds SLABW/IBW of them, exp'd by one ACT op.
    cpg = SLABW // IBW
    groups = []
    c = 0
    while c < NT:
        n = min(cpg, NT - c)
        groups.append((c, n))
        c += n

    with tile.TileContext(nc) as tc, ExitStack() as ctx:
        singles = ctx.enter_context(tc.tile_pool(name="singles", bufs=1))
        nat = ctx.enter_context(tc.tile_pool(name="nat", bufs=2))
        stats = ctx.enter_context(tc.tile_pool(name="stats", bufs=2))
        bigT = ctx.enter_context(tc.tile_pool(name="bigT", bufs=2))
        et_pool = ctx.enter_context(tc.tile_pool(name="et", bufs=6))
        osb_pool = ctx.enter_context(tc.tile_pool(name="osb", bufs=2))
        zr_pool = ctx.enter_context(tc.tile_pool(name="zr", bufs=4))
        out_pool = ctx.enter_context(tc.tile_pool(name="outp", bufs=2))

        ident = singles.tile([128, 128], F32)
        make_identity(nc, ident)
        qs_sb = singles.tile([128, H_PER_CORE], F32)
        nc.sync.dma_start(qs_sb[:], qs_d[:, :])

        QT2, KT2, VX = [], [], []

        # ---------------- PREP: both heads (no PE/PSUM use at all --
        # transposes go through the DMA xbar, so prep overlaps main freely).
        # K path is emitted first: every stage-1 matmul needs all of KT2,
        # while QT2 is consumed i-block by i-block.
        from concourse.tile_rust import add_dep_helper
        gate = None   # last critical DVE inst of head 0, ordering hint
        for h in range(H_PER_CORE):
            qh = q_d[h].rearrange("(p t) d -> p t d", t=NT)
            kh = k_d[h].rearrange("(p t) d -> p t d", t=NT)
            vh = v_d[h].rearrange("(p t) d -> p t d", t=NT)

            q_nat = nat.tile([128, NT, D], F32, tag="qnat")
            k_nat = nat.tile([128, NT, D], F32, tag="knat")
            nc.sync.dma_start(k_nat[:], kh)
            nc.sync.dma_start(q_nat[:], qh)

            qt2 = bigT.tile([128, NT, 128], FP16, tag="qt2")
            kt2 = bigT.tile([128, NPAIR, 128], FP16, tag="kt2")
            vx = bigT.tile([128, NT, D + 1], FP16, tag="vx")
            QT2.append(qt2)
            KT2.append(kt2)
            VX.append(vx)

            # V: fp16 via cast-DMA (SWDGE) + ones column
            nc.gpsimd.dma_start(vx[:, :, 0:D], vh)
            ones = stats.tile([128, NT], F32, tag="ones")
            nc.vector.memset(ones[:], 1.0)
            nc.vector.tensor_copy(
                vx[:, :, D:D + 1].rearrange("p t one -> p (t one)"), ones[:])

            # ---- K path ----
            sk = stats.tile([128, NT, D], F32, tag="sk")
            ssk = stats.tile([128, NT], F32, tag="ssk")
            if h == 0:
                nc.scalar.activation(sk[:], k_nat[:],
                                     mybir.ActivationFunctionType.Square)
            else:
                i0 = nc.vector.tensor_mul(sk[:], k_nat[:], k_nat[:])
                if gate is not None:
                    add_dep_helper(i0.ins, gate.ins, sync=False,
                                   reason="head1 prep after head0 critical path")
            nc.vector.tensor_reduce(
                ssk[:].rearrange("p (t one) -> p t one", one=1), sk[:],
                axis=mybir.AxisListType.X, op=mybir.AluOpType.add)
            rk = _newton_rsqrt(nc, stats, ssk[:], NT)
            kn_all = nat.tile([128, NT, D], FP16, tag="knall")
            nc.vector.tensor_mul(
                kn_all[:], k_nat[:],
                rk.rearrange("p (t one) -> p t one", one=1)
                .to_broadcast([128, NT, D]))
            for a4 in range(0, NPAIR, 8):
                na4 = min(8, NPAIR - a4)
                nc.sync.dma_start_transpose(
                    kt2[:, a4:a4 + na4, :],
                    kn_all[:, 2 * a4:2 * (a4 + na4), :].rearrange("p t d -> p (t d)"))

            # ---- Q path ----
            sq = stats.tile([128, NT, D], F32, tag="sq")
            ssq = stats.tile([128, NT], F32, tag="ssq")
            if h == 0:
                nc.scalar.activation(sq[:], q_nat[:],
                                     mybir.ActivationFunctionType.Square)
            else:
                nc.vector.tensor_mul(sq[:], q_nat[:], q_nat[:])
            nc.vector.tensor_reduce(
                ssq[:].rearrange("p (t one) -> p t one", one=1), sq[:],
                axis=mybir.AxisListType.X, op=mybir.AluOpType.add)
            rq = _newton_rsqrt(nc, stats, ssq[:], NT)
            # fold per-head logit scale into the q side
            nc.vector.tensor_scalar_mul(rq, rq, qs_sb[:, h:h + 1])
            qn2_all = nat.tile([128, NT, 2, D], FP16, tag="qnall")
            rq_b = (rq.rearrange("p (t one) -> p t one", one=1)
                    .to_broadcast([128, NT, D]))
            nc.vector.tensor_mul(qn2_all[:, :, 0, :], q_nat[:], rq_b)
            g = nc.vector.tensor_mul(qn2_all[:, :, 1, :], q_nat[:], rq_b)
            if h == 0:
                gate = g
            for t4 in range(0, NT, 8):
                nt4 = min(8, NT - t4)
                nc.sync.dma_start_transpose(
                    qt2[:, t4:t4 + nt4, :],
                    qn2_all[:, t4:t4 + nt4, :, :].rearrange("p t a d -> p (t a d)"))

        # ---------------- MAIN: per head ----------------
        ps_slab = ctx.enter_context(tc.tile_pool(name="ps_slab", bufs=2, space="PSUM"))
        ps_o = ctx.enter_context(tc.tile_pool(name="ps_o", bufs=1, space="PSUM"))
        ps_ot = ctx.enter_context(tc.tile_pool(name="ps_ot", bufs=1, space="PSUM"))
        def emit_drain(pend):
            """Drain a finished o_ps accumulator: copy to SBUF, PE-transpose
            each 128-column chunk back to row-major, multiply by 1/Z."""
            o_ps, out_sb, ib = pend
            o_sb = osb_pool.tile([65, IBW], F32, tag="osb")
            nc.vector.tensor_copy(o_sb[:], o_ps[:])
            for tchunk in range(CPI):
                otp = ps_ot.tile([128, D + 1], F32, tag="otp")
                nc.tensor.transpose(
                    otp[:], o_sb[:, tchunk * 128:(tchunk + 1) * 128],
                    ident[0:65, 0:65])
                zr = zr_pool.tile([128, 1], F32, tag="zrt")
                nc.vector.reciprocal_approx_fast(zr[:], otp[:, D:D + 1])
                nc.vector.tensor_scalar_mul(
                    out_sb[:, CPI * ib + tchunk, :], otp[:, 0:D], zr[:])

        pending = None
        OUT_SB = []
        for h in range(H_PER_CORE):
            qt2, kt2, vx = QT2[h], KT2[h], VX[h]
            out_sb = out_pool.tile([128, NT, D], F32, tag="outsb")
            OUT_SB.append(out_sb)
            for ib in range(NIB):
                rhsA = qt2[0:64, CPI * ib:CPI * (ib + 1), :].rearrange("p a b -> p (a b)")
                rhsB = qt2[64:128, CPI * ib:CPI * (ib + 1), :].rearrange("p a b -> p (a b)")
                o_ps = ps_o.tile([65, IBW], F32, tag="ops")
                for gi, (c0, ng) in enumerate(groups):
                    slab = ps_slab.tile([128, SLABW], F32, tag="slab")
                    for cc in range(ng):
                        c = c0 + cc
                        if c % 2 == 0:
                            nc.tensor.matmul(
                                slab[:, cc * IBW:(cc + 1) * IBW],
                                kt2[0:64, c // 2, :], rhsA,
                                start=True, stop=True, tile_position=(0, 0))
                        else:
                            nc.tensor.matmul(
                                slab[:, cc * IBW:(cc + 1) * IBW],
                                kt2[64:128, c // 2, :], rhsB,
                                start=True, stop=True, tile_position=(64, 0))
                    if gi == 0 and pending is not None:
                        # drain the previous i-block only after this one's
                        # first slab fill is queued, so PE never stalls on it
                        emit_drain(pending)
                        pending = None
                    et = et_pool.tile([128, SLABW], FP16, tag="et")
                    if gi in DVE_GROUPS:
                        nc.vector.tensor_scalar(
                            out=et.bitcast(I16)[:, 0:ng * IBW],
                            in0=slab[:, 0:ng * IBW],
                            scalar1=A_SCH, scalar2=B_SCH,
                            op0=mybir.AluOpType.mult, op1=mybir.AluOpType.add)
                    elif gi == len(groups) - 1 and ng == 2:
                        # split the final partial slab: ACT + DVE half each
                        nc.scalar.activation(et[:, 0:IBW], slab[:, 0:IBW],
                                             mybir.ActivationFunctionType.Exp)
                        nc.vector.tensor_scalar(
                            out=et.bitcast(I16)[:, IBW:2 * IBW],
                            in0=slab[:, IBW:2 * IBW],
                            scalar1=A_SCH, scalar2=B_SCH,
                            op0=mybir.AluOpType.mult, op1=mybir.AluOpType.add)
                    else:
                        nc.scalar.activation(et[:, 0:ng * IBW], slab[:, 0:ng * IBW],
                                             mybir.ActivationFunctionType.Exp)
                    for cc in range(ng):
                        c = c0 + cc
                        nc.tensor.matmul(
                            o_ps[:], vx[:, c, :], et[:, cc * IBW:(cc + 1) * IBW],
                            start=(c == 0), stop=(c == NT - 1),
                            skip_group_check=True)
                pending = (o_ps, out_sb, ib)
            # out DMA emitted after the head's last drain (see below)
        emit_drain(pending)
        pending = None
        for h in range(H_PER_CORE):
            nc.sync.dma_start(
                o_d[h].rearrange("(p t) d -> p t d", t=NT), OUT_SB[h][:])

    nc.compile()
    return nc


_NC_CACHE = {}
TRACE = False        # set by test harness for profiling runs
LAST_RESULT = None   # BassKernelResults of the most recent kernel() call


def _get_nc(S):
    if S not in _NC_CACHE:
        _NC_CACHE[S] = build_kernel(S)
    return _NC_CACHE[S]


def kernel(queries, keys, values, logit_scale):
    B, H, S, D_ = queries.shape
    assert B == 1 and D_ == D and H == H_TOTAL
    nc = _get_nc(S)

    # host-side: per-head scale = exp(min(logit_scale, ln sqrt(D)))
    scale = np.exp(np.minimum(np.asarray(logit_scale, np.float32).reshape(H),
                              math.log(math.sqrt(D)))).astype(np.float32)

    in_maps = []
    for c in range(N_CORES):
        h0 = c * H_PER_CORE
        sl = slice(h0, h0 + H_PER_CORE)
        in_maps.append({
            "q": np.ascontiguousarray(queries[0, sl]),
            "k": np.ascontiguousarray(keys[0, sl]),
            "v": np.ascontiguousarray(values[0, sl]),
            "qs": np.ascontiguousarray(
                np.broadcast_to(scale[sl][None, :], (128, H_PER_CORE))),
        })

    res = bass_utils.run_bass_kernel_spmd(
        nc, in_maps, core_ids=list(range(N_CORES)), trace=TRACE)
    global LAST_RESULT
    LAST_RESULT = res

    out = np.empty((B, H, S, D), np.float32)
    for c in range(N_CORES):
        out[0, c * H_PER_CORE:(c + 1) * H_PER_CORE] = res.results[c]["o"]
    return out



# revision 9
# speedup vs baseline: 1.3782x; 1.2510x over previous
"""Cosine attention (B=1, H=16, S=4096, D=64) on 8 trn2 NeuronCores.

Sharding: batch*heads split across cores -> 2 heads per core, full seq per
head (softmax is per-row, no cross-core communication).

Per-head pipeline on a core:
  1. load Q,K,V [4096,64] in "(p t) d" layout -> [128, 32, 64]
  2. row sumsq (ACT Square + DVE reduce), rsqrt via Newton iterations (DVE
     bit-trick, no ACT table switches), fold exp(min(logit_scale, ln 8))
     into the Q-side scale
  3. normalize (broadcast tensor_tensor, fp16) + DMA-xbar transpose to
     d-major fp16: QT2 [128, 32, 128] (partition halves both hold Qt),
     KT2 [128, 16, 128] (even chunks in partitions 0-63, odd in 64-127)
  4. stage 1: row-packed fp16 matmul pairs S^T[j-chunk, i-block] -> PSUM
     slabs [128, 1536] (ping-pong)
  5. exp on ScalarE straight from PSUM -> E^T tiles (fp16)
  6. stage 2 (fp16): o_ps[65, 512] += Vx[chunk].T @ E^T (Vx has a ones
     column so row 64 accumulates the softmax denominator Z)
  7. drain: copy to SBUF, PE-transpose back to row-major, multiply by
     1/Z (fast reciprocal), one 1 MB DMA out per head.
"""

import math
from contextlib import ExitStack

import numpy as np

import concourse.bass as bass
import concourse.tile as tile
from concourse import bacc, mybir
import concourse.bass_utils as bass_utils

F32 = mybir.dt.float32
F32R = mybir.dt.float32r
BF16 = mybir.dt.bfloat16
FP16 = mybir.dt.float16
I32 = mybir.dt.int32
I16 = mybir.dt.int16

N_CORES = 8
H_TOTAL = 16
H_PER_CORE = H_TOTAL // N_CORES
D = 64
IBW = 512          # i-block width (PSUM bank / fp32 moving-operand limit)
SLABW = 1536       # exp slab width (3 PSUM banks)

# Schraudolph fast-exp on DVE: i16 = round(x*A + B), bitcast as fp16 gives
# exp(x)*(1+eps), |eps|<~3.5%. Softmax cancels the common-mode part; the
# residual output error for a ~1/3 share of entries is ~8e-3 (simulated).
A_SCH = 1024.0 * 1.4426950408889634
B_SCH = 15.0 * 1024.0 - 45.0
DVE_GROUPS = (2, 5, 8, 9)  # slab groups exp'd on DVE; group 10 is split


def _newton_rsqrt(nc, pool, ss, n):
    """rsqrt of ss [128, n] (fp32, positive) via bit-trick seed + 3 Newton
    iterations, all on VectorE. Returns a [128, n] fp32 AP."""
    seed_i = pool.tile([128, n], I32, tag="nw_i")
    # ~(i >> 1)
    nc.vector.tensor_scalar(
        out=seed_i[:], in0=ss.bitcast(I32), scalar1=1, scalar2=-1,
        op0=mybir.AluOpType.logical_shift_right, op1=mybir.AluOpType.bitwise_xor)
    # + (0x5f3759df + 1)  == 0x5f3759df - (i >> 1)
    nc.vector.tensor_scalar(
        out=seed_i[:], in0=seed_i[:], scalar1=0x5F3759E0, scalar2=None,
        op0=mybir.AluOpType.add)
    y = seed_i.bitcast(F32)
    t = pool.tile([128, n], F32, tag="nw_t")
    for _ in range(3):
        nc.vector.tensor_mul(t[:], y, y)            # y*y
        nc.vector.tensor_mul(t[:], t[:], ss)        # x*y*y
        nc.vector.tensor_scalar(                    # 1.5 - 0.5*x*y*y
            out=t[:], in0=t[:], scalar1=-0.5, scalar2=1.5,
            op0=mybir.AluOpType.mult, op1=mybir.AluOpType.add)
        nc.vector.tensor_mul(y, y, t[:])            # y *= t
    return y


def build_kernel(S):
    """Build the 2-head-per-core cosine attention program for seq len S."""
    NT = S // 128            # 128-row tiles per head
    NPAIR = NT // 2
    NIB = S // IBW           # i-blocks
    CPI = IBW // 128         # output chunks per i-block

    nc = bacc.Bacc("TRN2", target_bir_lowering=False, debug=False,
                   enable_asserts=False, num_devices=N_CORES)

    q_d = nc.dram_tensor("q", [H_PER_CORE, S, D], F32, kind="ExternalInput").ap()
    k_d = nc.dram_tensor("k", [H_PER_CORE, S, D], F32, kind="ExternalInput").ap()
    v_d = nc.dram_tensor("v", [H_PER_CORE, S, D], F32, kind="ExternalInput").ap()
    qs_d = nc.dram_tensor("qs", [128, H_PER_CORE], F32, kind="ExternalInput").ap()
    o_d = nc.dram_tensor("o", [H_PER_CORE, S, D], F32, kind="ExternalOutput").ap()

    # Each stage-1 matmul output S^T[j-chunk, i-block] is [128, IBW] = one
    # PSUM bank; a slab holds SLABW/IBW of them, exp'd by one ACT op.
    cpg = SLABW // IBW
    groups = []
    c = 0
    while c < NT:
        n = min(cpg, NT - c)
        groups.append((c, n))
        c += n

    with tile.TileContext(nc) as tc, ExitStack() as ctx:
        singles = ctx.enter_context(tc.tile_pool(name="singles", bufs=1))
        nat = ctx.enter_context(tc.tile_pool(name="nat", bufs=2))
        stats = ctx.enter_context(tc.tile_pool(name="stats", bufs=2))
        bigT = ctx.enter_context(tc.tile_pool(name="bigT", bufs=2))
        et_pool = ctx.enter_context(tc.tile_pool(name="et", bufs=6))
        zr_pool = ctx.enter_context(tc.tile_pool(name="zr", bufs=4))
        out_pool = ctx.enter_context(tc.tile_pool(name="outp", bufs=2))

        qs_sb = singles.tile([128, H_PER_CORE], F32)
        nc.sync.dma_start(qs_sb[:], qs_d[:, :])

        QT2, KT2, VX = [], [], []

        # ---------------- PREP: both heads (no PE/PSUM use at all --
        # transposes go through the DMA xbar, so prep overlaps main freely).
        # K path is emitted first: every stage-1 matmul needs all of KT2,
        # while QT2 is consumed i-block by i-block.
        from concourse.tile_rust import add_dep_helper
        gate = None   # last critical DVE inst of head 0, ordering hint
        for h in range(H_PER_CORE):
            qh = q_d[h].rearrange("(p t) d -> p t d", t=NT)
            kh = k_d[h].rearrange("(p t) d -> p t d", t=NT)
            vh = v_d[h].rearrange("(p t) d -> p t d", t=NT)

            q_nat = nat.tile([128, NT, D], F32, tag="qnat")
            k_nat = nat.tile([128, NT, D], F32, tag="knat")
            nc.sync.dma_start(k_nat[:], kh)
            nc.sync.dma_start(q_nat[:], qh)

            qt2 = bigT.tile([128, NT, 128], FP16, tag="qt2")
            kt2 = bigT.tile([128, NPAIR, 128], FP16, tag="kt2")
            vx = bigT.tile([128, NT, D + 1], FP16, tag="vx")
            QT2.append(qt2)
            KT2.append(kt2)
            VX.append(vx)

            # V: fp16 via cast-DMA (SWDGE) + ones column
            nc.gpsimd.dma_start(vx[:, :, 0:D], vh)
            ones = stats.tile([128, NT], F32, tag="ones")
            nc.vector.memset(ones[:], 1.0)
            nc.vector.tensor_copy(
                vx[:, :, D:D + 1].rearrange("p t one -> p (t one)"), ones[:])

            # ---- K path ----
            sk = stats.tile([128, NT, D], F32, tag="sk")
            ssk = stats.tile([128, NT], F32, tag="ssk")
            if h == 0:
                nc.scalar.activation(sk[:], k_nat[:],
                                     mybir.ActivationFunctionType.Square)
            else:
                i0 = nc.vector.tensor_mul(sk[:], k_nat[:], k_nat[:])
                if gate is not None:
                    add_dep_helper(i0.ins, gate.ins, sync=False,
                                   reason="head1 prep after head0 critical path")
            nc.vector.tensor_reduce(
                ssk[:].rearrange("p (t one) -> p t one", one=1), sk[:],
                axis=mybir.AxisListType.X, op=mybir.AluOpType.add)
            rk = _newton_rsqrt(nc, stats, ssk[:], NT)
            kn_all = nat.tile([128, NT, D], FP16, tag="knall")
            nc.vector.tensor_mul(
                kn_all[:], k_nat[:],
                rk.rearrange("p (t one) -> p t one", one=1)
                .to_broadcast([128, NT, D]))
            for a4 in range(0, NPAIR, 8):
                na4 = min(8, NPAIR - a4)
                nc.sync.dma_start_transpose(
                    kt2[:, a4:a4 + na4, :],
                    kn_all[:, 2 * a4:2 * (a4 + na4), :].rearrange("p t d -> p (t d)"))

            # ---- Q path ----
            sq = stats.tile([128, NT, D], F32, tag="sq")
            ssq = stats.tile([128, NT], F32, tag="ssq")
            if h == 0:
                nc.scalar.activation(sq[:], q_nat[:],
                                     mybir.ActivationFunctionType.Square)
            else:
                nc.vector.tensor_mul(sq[:], q_nat[:], q_nat[:])
            nc.vector.tensor_reduce(
                ssq[:].rearrange("p (t one) -> p t one", one=1), sq[:],
                axis=mybir.AxisListType.X, op=mybir.AluOpType.add)
            rq = _newton_rsqrt(nc, stats, ssq[:], NT)
            # fold per-head logit scale into the q side
            nc.vector.tensor_scalar_mul(rq, rq, qs_sb[:, h:h + 1])
            qn2_all = nat.tile([128, NT, 2, D], FP16, tag="qnall")
            rq_b = (rq.rearrange("p (t one) -> p t one", one=1)
                    .to_broadcast([128, NT, D]))
            nc.vector.tensor_mul(qn2_all[:, :, 0, :], q_nat[:], rq_b)
            g = nc.vector.tensor_mul(qn2_all[:, :, 1, :], q_nat[:], rq_b)
            if h == 0:
                gate = g
            for t4 in range(0, NT, 8):
                nt4 = min(8, NT - t4)
                nc.sync.dma_start_transpose(
                    qt2[:, t4:t4 + nt4, :],
                    qn2_all[:, t4:t4 + nt4, :, :].rearrange("p t a d -> p (t a d)"))

        # ---------------- MAIN: software-pipelined slab stream ----------
        # Tasks are slabs across (head, i-block, group). Emission order is
        # skewed: S1(t), EXP(t-1), S2(t-2) -- so the in-order PE queue never
        # holds a stage-2 matmul (waiting on exp) in front of runnable
        # stage-1 work, and exp engines stream back to back.
        #
        # Stage 2 is the "O-form": E^T tiles are the stationary operand
        # (fp16, 128 cols -> fast weight load) and Vx streams [128, 65].
        # out[i, 0:64] = O row-chunk (row-major!), out[i, 64] = Z. No PE
        # transpose and no PSUM copy in the drain; o_ps double-buffers.
        ps_slab = ctx.enter_context(tc.tile_pool(name="ps_slab", bufs=2, space="PSUM"))
        ps_o = ctx.enter_context(tc.tile_pool(name="ps_o", bufs=2, space="PSUM"))
        OW = 66     # o_ps per-subchunk stride (8B-aligned)

        tasks = []
        for h in range(H_PER_CORE):
            for ib in range(NIB):
                for gi, (c0, ng) in enumerate(groups):
                    tasks.append((h, ib, gi, c0, ng))
        T = len(tasks)
        ngroups = len(groups)

        out_sb = [out_pool.tile([128, NT, D], F32, tag="outsb",
                                name=f"out_sb{h}")
                  for h in range(H_PER_CORE)]
        slabs = {}   # live slab tiles by task index
        ets = {}     # live E^T tiles by task index
        o_ps_cur = [None]   # accumulator for the i-block being stage-2'd

        def emit_s1(t):
            h, ib, gi, c0, ng = tasks[t]
            qt2, kt2 = QT2[h], KT2[h]
            rhsA = qt2[0:64, CPI * ib:CPI * (ib + 1), :].rearrange("p a b -> p (a b)")
            rhsB = qt2[64:128, CPI * ib:CPI * (ib + 1), :].rearrange("p a b -> p (a b)")
            slab = ps_slab.tile([128, SLABW], F32, tag="slab")
            slabs[t] = slab
            for cc in range(ng):
                c = c0 + cc
                if c % 2 == 0:
                    nc.tensor.matmul(
                        slab[:, cc * IBW:(cc + 1) * IBW],
                        kt2[0:64, c // 2, :], rhsA,
                        start=True, stop=True, tile_position=(0, 0))
                else:
                    nc.tensor.matmul(
                        slab[:, cc * IBW:(cc + 1) * IBW],
                        kt2[64:128, c // 2, :], rhsB,
                        start=True, stop=True, tile_position=(64, 0))

        def emit_exp(t):
            h, ib, gi, c0, ng = tasks[t]
            slab = slabs.pop(t)
            et = et_pool.tile([128, SLABW], FP16, tag="et")
            ets[t] = et
            if gi in DVE_GROUPS:
                nc.vector.tensor_scalar(
                    out=et.bitcast(I16)[:, 0:ng * IBW],
                    in0=slab[:, 0:ng * IBW],
                    scalar1=A_SCH, scalar2=B_SCH,
                    op0=mybir.AluOpType.mult, op1=mybir.AluOpType.add)
            elif gi == ngroups - 1 and ng == 2:
                # split the final partial slab: ACT + DVE half each
                nc.scalar.activation(et[:, 0:IBW], slab[:, 0:IBW],
                                     mybir.ActivationFunctionType.Exp)
                nc.vector.tensor_scalar(
                    out=et.bitcast(I16)[:, IBW:2 * IBW],
                    in0=slab[:, IBW:2 * IBW],
                    scalar1=A_SCH, scalar2=B_SCH,
                    op0=mybir.AluOpType.mult, op1=mybir.AluOpType.add)
            else:
                nc.scalar.activation(et[:, 0:ng * IBW], slab[:, 0:ng * IBW],
                                     mybir.ActivationFunctionType.Exp)

        def emit_s2(t):
            h, ib, gi, c0, ng = tasks[t]
            vx = VX[h]
            et = ets.pop(t)
            if gi == 0:
                o_ps_cur[0] = ps_o.tile([128, CPI, OW], F32, tag="ops", name="o_ps")
            o_ps = o_ps_cur[0]
            for cc in range(ng):
                c = c0 + cc
                for ic in range(CPI):
                    # start only on the bank's very first matmul: first_mm
                    # clears has_written for the WHOLE bank, so a start on
                    # each sub-chunk would wipe the others' partials.
                    nc.tensor.matmul(
                        o_ps[:, ic, 0:D + 1],
                        et[:, cc * IBW + ic * 128:cc * IBW + (ic + 1) * 128],
                        vx[:, c, :],
                        start=(c == 0 and ic == 0), stop=(c == NT - 1),
                        skip_group_check=True)
            if gi == ngroups - 1:
                # drain: out rows are already row-major; divide by Z
                zr = zr_pool.tile([128, CPI], F32, tag="zrt")
                nc.vector.reciprocal_approx_fast(
                    zr[:], o_ps[:, :, D:D + 1].rearrange("p a one -> p (a one)"))
                for ic in range(CPI):
                    nc.vector.tensor_scalar_mul(
                        out_sb[h][:, CPI * ib + ic, :], o_ps[:, ic, 0:D],
                        zr[:, ic:ic + 1])
                if ib == NIB - 1:
                    nc.sync.dma_start(
                        o_d[h].rearrange("(p t) d -> p t d", t=NT), out_sb[h][:])

        for t in range(T + 2):
            if t < T:
                emit_s1(t)
            if 0 <= t - 1 < T:
                emit_exp(t - 1)
            if 0 <= t - 2 < T:
                emit_s2(t - 2)

    nc.compile()
    return nc


_NC_CACHE = {}
TRACE = False        # set by test harness for profiling runs
LAST_RESULT = None   # BassKernelResults of the most recent kernel() call


def _get_nc(S):
    if S not in _NC_CACHE:
        _NC_CACHE[S] = build_kernel(S)
    return _NC_CACHE[S]


def kernel(queries, keys, values, logit_scale):
    B, H, S, D_ = queries.shape
    assert B == 1 and D_ == D and H == H_TOTAL
    nc = _get_nc(S)

    # host-side: per-head scale = exp(min(logit_scale, ln sqrt(D)))
    scale = np.exp(np.minimum(np.asarray(logit_scale, np.float32).reshape(H),
                              math.log(math.sqrt(D)))).astype(np.float32)

    in_maps = []
    for c in range(N_CORES):
        h0 = c * H_PER_CORE
        sl = slice(h0, h0 + H_PER_CORE)
        in_maps.append({
            "q": np.ascontiguousarray(queries[0, sl]),
            "k": np.ascontiguousarray(keys[0, sl]),
            "v": np.ascontiguousarray(values[0, sl]),
            "qs": np.ascontiguousarray(
                np.broadcast_to(scale[sl][None, :], (128, H_PER_CORE))),
        })

    res = bass_utils.run_bass_kernel_spmd(
        nc, in_maps, core_ids=list(range(N_CORES)), trace=TRACE)
    global LAST_RESULT
    LAST_RESULT = res

    out = np.empty((B, H, S, D), np.float32)
    for c in range(N_CORES):
        out[0, c * H_PER_CORE:(c + 1) * H_PER_CORE] = res.results[c]["o"]
    return out



# revision 10
# speedup vs baseline: 1.5321x; 1.1117x over previous
"""Cosine attention (B=1, H=16, S=4096, D=64) on 8 trn2 NeuronCores.

Sharding: batch*heads split across cores -> 2 heads per core, full seq per
head (softmax is per-row, no cross-core communication).

Per-head pipeline on a core:
  1. load Q,K,V [4096,64] in "(p t) d" layout -> [128, 32, 64]
  2. row sumsq (ACT Square + DVE reduce), rsqrt via Newton iterations (DVE
     bit-trick, no ACT table switches), fold exp(min(logit_scale, ln 8))
     into the Q-side scale
  3. normalize (broadcast tensor_tensor, fp16) + DMA-xbar transpose to
     d-major fp16: QT2 [128, 32, 128] (partition halves both hold Qt),
     KT2 [128, 16, 128] (even chunks in partitions 0-63, odd in 64-127)
  4. stage 1: row-packed fp16 matmul pairs S^T[j-chunk, i-block] -> PSUM
     slabs [128, 1536] (ping-pong)
  5. exp on ScalarE straight from PSUM -> E^T tiles (fp16)
  6. stage 2 (fp16): o_ps[65, 512] += Vx[chunk].T @ E^T (Vx has a ones
     column so row 64 accumulates the softmax denominator Z)
  7. drain: copy to SBUF, PE-transpose back to row-major, multiply by
     1/Z (fast reciprocal), one 1 MB DMA out per head.
"""

import math
from contextlib import ExitStack

import numpy as np

import concourse.bass as bass
import concourse.tile as tile
from concourse import bacc, mybir
import concourse.bass_utils as bass_utils

F32 = mybir.dt.float32
F32R = mybir.dt.float32r
BF16 = mybir.dt.bfloat16
FP16 = mybir.dt.float16
I32 = mybir.dt.int32
I16 = mybir.dt.int16

N_CORES = 8
H_TOTAL = 16
H_PER_CORE = H_TOTAL // N_CORES
D = 64
IBW = 512          # i-block width (PSUM bank / fp32 moving-operand limit)
SLABW = 1024       # exp slab width (2 PSUM banks; 3-deep window)

# Schraudolph fast-exp on DVE: i16 = round(x*A + B), bitcast as fp16 gives
# exp(x)*(1+eps), |eps|<~3.5%. Softmax cancels the common-mode part; the
# residual output error for a ~1/3 share of entries is ~8e-3 (simulated).
A_SCH = 1024.0 * 1.4426950408889634
B_SCH = 15.0 * 1024.0 - 45.0
DVE_GROUPS = (1, 4, 6, 9, 11, 14)  # slab groups (of 16) exp'd on DVE


def _newton_rsqrt(nc, pool, ss, n):
    """rsqrt of ss [128, n] (fp32, positive) via bit-trick seed + 3 Newton
    iterations, all on VectorE. Returns a [128, n] fp32 AP."""
    seed_i = pool.tile([128, n], I32, tag="nw_i")
    # ~(i >> 1)
    nc.vector.tensor_scalar(
        out=seed_i[:], in0=ss.bitcast(I32), scalar1=1, scalar2=-1,
        op0=mybir.AluOpType.logical_shift_right, op1=mybir.AluOpType.bitwise_xor)
    # + (0x5f3759df + 1)  == 0x5f3759df - (i >> 1)
    nc.vector.tensor_scalar(
        out=seed_i[:], in0=seed_i[:], scalar1=0x5F3759E0, scalar2=None,
        op0=mybir.AluOpType.add)
    y = seed_i.bitcast(F32)
    t = pool.tile([128, n], F32, tag="nw_t")
    for _ in range(3):
        nc.vector.tensor_mul(t[:], y, y)            # y*y
        nc.vector.tensor_mul(t[:], t[:], ss)        # x*y*y
        nc.vector.tensor_scalar(                    # 1.5 - 0.5*x*y*y
            out=t[:], in0=t[:], scalar1=-0.5, scalar2=1.5,
            op0=mybir.AluOpType.mult, op1=mybir.AluOpType.add)
        nc.vector.tensor_mul(y, y, t[:])            # y *= t
    return y


def build_kernel(S):
    """Build the 2-head-per-core cosine attention program for seq len S."""
    NT = S // 128            # 128-row tiles per head
    NPAIR = NT // 2
    NIB = S // IBW           # i-blocks
    CPI = IBW // 128         # output chunks per i-block

    nc = bacc.Bacc("TRN2", target_bir_lowering=False, debug=False,
                   enable_asserts=False, num_devices=N_CORES)

    q_d = nc.dram_tensor("q", [H_PER_CORE, S, D], F32, kind="ExternalInput").ap()
    k_d = nc.dram_tensor("k", [H_PER_CORE, S, D], F32, kind="ExternalInput").ap()
    v_d = nc.dram_tensor("v", [H_PER_CORE, S, D], F32, kind="ExternalInput").ap()
    qs_d = nc.dram_tensor("qs", [128, H_PER_CORE], F32, kind="ExternalInput").ap()
    o_d = nc.dram_tensor("o", [H_PER_CORE, S, D], F32, kind="ExternalOutput").ap()

    # Each stage-1 matmul output S^T[j-chunk, i-block] is [128, IBW] = one
    # PSUM bank; a slab holds SLABW/IBW of them, exp'd by one ACT op.
    cpg = SLABW // IBW
    groups = []
    c = 0
    while c < NT:
        n = min(cpg, NT - c)
        groups.append((c, n))
        c += n

    with tile.TileContext(nc) as tc, ExitStack() as ctx:
        singles = ctx.enter_context(tc.tile_pool(name="singles", bufs=1))
        nat = ctx.enter_context(tc.tile_pool(name="nat", bufs=2))
        stats = ctx.enter_context(tc.tile_pool(name="stats", bufs=2))
        bigT = ctx.enter_context(tc.tile_pool(name="bigT", bufs=2))
        et_pool = ctx.enter_context(tc.tile_pool(name="et", bufs=6))
        zr_pool = ctx.enter_context(tc.tile_pool(name="zr", bufs=4))
        out_pool = ctx.enter_context(tc.tile_pool(name="outp", bufs=2))

        qs_sb = singles.tile([128, H_PER_CORE], F32)
        nc.sync.dma_start(qs_sb[:], qs_d[:, :])

        QT2, KT2, VX = [], [], []

        # ---------------- PREP: both heads (no PE/PSUM use at all --
        # transposes go through the DMA xbar, so prep overlaps main freely).
        # K path is emitted first: every stage-1 matmul needs all of KT2,
        # while QT2 is consumed i-block by i-block.
        from concourse.tile_rust import add_dep_helper
        gate = None   # last critical DVE inst of head 0, ordering hint
        for h in range(H_PER_CORE):
            qh = q_d[h].rearrange("(p t) d -> p t d", t=NT)
            kh = k_d[h].rearrange("(p t) d -> p t d", t=NT)
            vh = v_d[h].rearrange("(p t) d -> p t d", t=NT)

            q_nat = nat.tile([128, NT, D], F32, tag="qnat")
            k_nat = nat.tile([128, NT, D], F32, tag="knat")
            nc.sync.dma_start(k_nat[:], kh)
            nc.sync.dma_start(q_nat[:], qh)

            qt2 = bigT.tile([128, NT, 128], FP16, tag="qt2")
            kt2 = bigT.tile([128, NPAIR, 128], FP16, tag="kt2")
            vx = bigT.tile([128, NT, D + 1], FP16, tag="vx")
            QT2.append(qt2)
            KT2.append(kt2)
            VX.append(vx)

            # V: fp16 via cast-DMA (SWDGE) + ones column
            nc.gpsimd.dma_start(vx[:, :, 0:D], vh)
            ones = stats.tile([128, NT], F32, tag="ones")
            nc.vector.memset(ones[:], 1.0)
            nc.vector.tensor_copy(
                vx[:, :, D:D + 1].rearrange("p t one -> p (t one)"), ones[:])

            # ---- K path ----
            sk = stats.tile([128, NT, D], F32, tag="sk")
            ssk = stats.tile([128, NT], F32, tag="ssk")
            if h == 0:
                nc.scalar.activation(sk[:], k_nat[:],
                                     mybir.ActivationFunctionType.Square)
            else:
                i0 = nc.vector.tensor_mul(sk[:], k_nat[:], k_nat[:])
                if gate is not None:
                    add_dep_helper(i0.ins, gate.ins, sync=False,
                                   reason="head1 prep after head0 critical path")
            nc.vector.tensor_reduce(
                ssk[:].rearrange("p (t one) -> p t one", one=1), sk[:],
                axis=mybir.AxisListType.X, op=mybir.AluOpType.add)
            rk = _newton_rsqrt(nc, stats, ssk[:], NT)
            kn_all = nat.tile([128, NT, D], FP16, tag="knall")
            nc.vector.tensor_mul(
                kn_all[:], k_nat[:],
                rk.rearrange("p (t one) -> p t one", one=1)
                .to_broadcast([128, NT, D]))
            for a4 in range(0, NPAIR, 8):
                na4 = min(8, NPAIR - a4)
                nc.sync.dma_start_transpose(
                    kt2[:, a4:a4 + na4, :],
                    kn_all[:, 2 * a4:2 * (a4 + na4), :].rearrange("p t d -> p (t d)"))

            # ---- Q path ----
            sq = stats.tile([128, NT, D], F32, tag="sq")
            ssq = stats.tile([128, NT], F32, tag="ssq")
            if h == 0:
                nc.scalar.activation(sq[:], q_nat[:],
                                     mybir.ActivationFunctionType.Square)
            else:
                nc.vector.tensor_mul(sq[:], q_nat[:], q_nat[:])
            nc.vector.tensor_reduce(
                ssq[:].rearrange("p (t one) -> p t one", one=1), sq[:],
                axis=mybir.AxisListType.X, op=mybir.AluOpType.add)
            rq = _newton_rsqrt(nc, stats, ssq[:], NT)
            # fold per-head logit scale into the q side
            nc.vector.tensor_scalar_mul(rq, rq, qs_sb[:, h:h + 1])
            qn2_all = nat.tile([128, NT, 2, D], FP16, tag="qnall")
            rq_b = (rq.rearrange("p (t one) -> p t one", one=1)
                    .to_broadcast([128, NT, D]))
            nc.vector.tensor_mul(qn2_all[:, :, 0, :], q_nat[:], rq_b)
            g = nc.vector.tensor_mul(qn2_all[:, :, 1, :], q_nat[:], rq_b)
            if h == 0:
                gate = g
            for t4 in range(0, NT, 8):
                nt4 = min(8, NT - t4)
                nc.sync.dma_start_transpose(
                    qt2[:, t4:t4 + nt4, :],
                    qn2_all[:, t4:t4 + nt4, :, :].rearrange("p t a d -> p (t a d)"))

        # ---------------- MAIN: software-pipelined slab stream ----------
        # Tasks are slabs across (head, i-block, group). Emission order is
        # skewed: S1(t), EXP(t-1), S2(t-2) -- so the in-order PE queue never
        # holds a stage-2 matmul (waiting on exp) in front of runnable
        # stage-1 work, and exp engines stream back to back.
        #
        # Stage 2 is the "O-form": E^T tiles are the stationary operand
        # (fp16, 128 cols -> fast weight load) and Vx streams [128, 65].
        # out[i, 0:64] = O row-chunk (row-major!), out[i, 64] = Z. No PE
        # transpose and no PSUM copy in the drain; o_ps double-buffers.
        ps_slab = ctx.enter_context(tc.tile_pool(name="ps_slab", bufs=3, space="PSUM"))
        ps_o = ctx.enter_context(tc.tile_pool(name="ps_o", bufs=2, space="PSUM"))
        OW = 66     # o_ps per-subchunk stride (8B-aligned)

        tasks = []
        for h in range(H_PER_CORE):
            for ib in range(NIB):
                for gi, (c0, ng) in enumerate(groups):
                    tasks.append((h, ib, gi, c0, ng))
        T = len(tasks)
        ngroups = len(groups)

        out_sb = [out_pool.tile([128, NT, D], F32, tag="outsb",
                                name=f"out_sb{h}")
                  for h in range(H_PER_CORE)]
        slabs = {}   # live slab tiles by task index
        ets = {}     # live E^T tiles by task index
        o_ps_cur = [None]   # accumulator for the i-block being stage-2'd

        def emit_s1(t):
            h, ib, gi, c0, ng = tasks[t]
            qt2, kt2 = QT2[h], KT2[h]
            rhsA = qt2[0:64, CPI * ib:CPI * (ib + 1), :].rearrange("p a b -> p (a b)")
            rhsB = qt2[64:128, CPI * ib:CPI * (ib + 1), :].rearrange("p a b -> p (a b)")
            slab = ps_slab.tile([128, SLABW], F32, tag="slab")
            slabs[t] = slab
            for cc in range(ng):
                c = c0 + cc
                if c % 2 == 0:
                    nc.tensor.matmul(
                        slab[:, cc * IBW:(cc + 1) * IBW],
                        kt2[0:64, c // 2, :], rhsA,
                        start=True, stop=True, tile_position=(0, 0))
                else:
                    nc.tensor.matmul(
                        slab[:, cc * IBW:(cc + 1) * IBW],
                        kt2[64:128, c // 2, :], rhsB,
                        start=True, stop=True, tile_position=(64, 0))

        def emit_exp(t):
            h, ib, gi, c0, ng = tasks[t]
            slab = slabs.pop(t)
            et = et_pool.tile([128, SLABW], FP16, tag="et")
            ets[t] = et
            if gi in DVE_GROUPS:
                nc.vector.tensor_scalar(
                    out=et.bitcast(I16)[:, 0:ng * IBW],
                    in0=slab[:, 0:ng * IBW],
                    scalar1=A_SCH, scalar2=B_SCH,
                    op0=mybir.AluOpType.mult, op1=mybir.AluOpType.add)
            else:
                nc.scalar.activation(et[:, 0:ng * IBW], slab[:, 0:ng * IBW],
                                     mybir.ActivationFunctionType.Exp)

        def emit_s2(t):
            h, ib, gi, c0, ng = tasks[t]
            vx = VX[h]
            et = ets.pop(t)
            if gi == 0:
                o_ps_cur[0] = ps_o.tile([128, CPI, OW], F32, tag="ops", name="o_ps")
            o_ps = o_ps_cur[0]
            for cc in range(ng):
                c = c0 + cc
                for ic in range(CPI):
                    # start only on the bank's very first matmul: first_mm
                    # clears has_written for the WHOLE bank, so a start on
                    # each sub-chunk would wipe the others' partials.
                    nc.tensor.matmul(
                        o_ps[:, ic, 0:D + 1],
                        et[:, cc * IBW + ic * 128:cc * IBW + (ic + 1) * 128],
                        vx[:, c, :],
                        start=(c == 0 and ic == 0), stop=(c == NT - 1),
                        skip_group_check=True)
            if gi == ngroups - 1:
                # drain: out rows are already row-major; divide by Z
                zr = zr_pool.tile([128, CPI], F32, tag="zrt")
                nc.vector.reciprocal_approx_fast(
                    zr[:], o_ps[:, :, D:D + 1].rearrange("p a one -> p (a one)"))
                for ic in range(CPI):
                    nc.vector.tensor_scalar_mul(
                        out_sb[h][:, CPI * ib + ic, :], o_ps[:, ic, 0:D],
                        zr[:, ic:ic + 1])
                if ib == NIB - 1:
                    nc.sync.dma_start(
                        o_d[h].rearrange("(p t) d -> p t d", t=NT), out_sb[h][:])

        for t in range(T + 2):
            if t < T:
                emit_s1(t)
            if 0 <= t - 1 < T:
                emit_exp(t - 1)
            if 0 <= t - 2 < T:
                emit_s2(t - 2)

    nc.compile()
    return nc


_NC_CACHE = {}
TRACE = False        # set by test harness for profiling runs
LAST_RESULT = None   # BassKernelResults of the most recent kernel() call


def _get_nc(S):
    if S not in _NC_CACHE:
        _NC_CACHE[S] = build_kernel(S)
    return _NC_CACHE[S]


def kernel(queries, keys, values, logit_scale):
    B, H, S, D_ = queries.shape
    assert B == 1 and D_ == D and H == H_TOTAL
    nc = _get_nc(S)

    # host-side: per-head scale = exp(min(logit_scale, ln sqrt(D)))
    scale = np.exp(np.minimum(np.asarray(logit_scale, np.float32).reshape(H),
                              math.log(math.sqrt(D)))).astype(np.float32)

    in_maps = []
    for c in range(N_CORES):
        h0 = c * H_PER_CORE
        sl = slice(h0, h0 + H_PER_CORE)
        in_maps.append({
            "q": np.ascontiguousarray(queries[0, sl]),
            "k": np.ascontiguousarray(keys[0, sl]),
            "v": np.ascontiguousarray(values[0, sl]),
            "qs": np.ascontiguousarray(
                np.broadcast_to(scale[sl][None, :], (128, H_PER_CORE))),
        })

    res = bass_utils.run_bass_kernel_spmd(
        nc, in_maps, core_ids=list(range(N_CORES)), trace=TRACE)
    global LAST_RESULT
    LAST_RESULT = res

    out = np.empty((B, H, S, D), np.float32)
    for c in range(N_CORES):
        out[0, c * H_PER_CORE:(c + 1) * H_PER_CORE] = res.results[c]["o"]
    return out

